# revision 14
# baseline (speedup 1.0000x reference)
"""Trainium2 Bass kernel for nn_MultiHeadDensityRatioEstimator.

Math restructure vs the jax reference:
  logits l_h(i,j) = -log1p(sq_h(i,j))  with sq = ||zy_i||^2+||zx_j||^2-2<zy_i,zx_j>
  exp(l_h) = 1/(1+sq_h) =: w_h   -> every logsumexp becomes a plain sum of w
  sum_h l_h = ln(prod_h w_h)     -> one log per pair instead of 8

Per core (8 cores, zy rows sharded 512/core):
  v_h = 1+sq_h from one K=18 augmented matmul per (head, tile)  [PSUM]
  w_h = reciprocal_approx_fast(v_h)                             [DVE]
  rowsums of w_h via ACT Copy+accum_out                         [ACT]
  savg = ln(prod_h w_h) stored [512,4096]                       [DVE/GPSIMD + ACT Ln]
  tiny AllReduce of the 8 per-head global sums -> baseline blavg
  sigmoid / count / sum sweeps over stored savg
  8 partial stats out per core; host combines to the 9 scalars.
"""

import math
import sys

import numpy as np

for _p in ("/opt/trn_rl_repo",):
    if _p not in sys.path:
        sys.path.insert(0, _p)

N = 4096
D = 128
H = 8
DH = 16
NCORES = 8
RPC = N // NCORES  # rows per core = 512
NIB = RPC // 128  # 4 i-blocks of 128 rows
NJB = N // 512  # 8 j-blocks of 512 cols
LOG_NN1 = float(np.log(float(N) * (N - 1)))
NSTAT = 8


def build_bass():
    import concourse.bacc as bacc
    import concourse.bass as bass
    import concourse.tile as tile
    from concourse import mybir

    f32 = mybir.dt.float32
    AF = mybir.ActivationFunctionType
    ALU = mybir.AluOpType
    AX = mybir.AxisListType

    nc = bacc.Bacc("TRN2", num_devices=NCORES, debug=False)

    zx = nc.dram_tensor("z_x", [N, D], f32, kind="ExternalInput")
    # z_yd[:, 0:128] = this core's zy rows; [:, 128:256] = matching zx rows
    zyd = nc.dram_tensor("z_yd", [RPC, 2 * D], f32, kind="ExternalInput")
    out = nc.dram_tensor("out", [1, NSTAT], f32, kind="ExternalOutput")

    from contextlib import ExitStack

    with tile.TileContext(nc) as tc, ExitStack() as stk:
        # ---------- persistent pools ----------
        big = stk.enter_context(tc.tile_pool(name="big", bufs=1))
        small = stk.enter_context(tc.tile_pool(name="small", bufs=1))

        # packed matmul operands: head h -> tensor HT[h], slot HS[h] (32-part
        # stride; matmul operand base partition must be 0/32/64)
        HT = [0, 0, 0, 1, 1, 1, 2, 2]
        HS = [0, 1, 2, 0, 1, 2, 0, 1]
        RHEADS = [[0, 1, 2], [3, 4, 5], [6, 7]]
        XTA = [big.tile([96, N], f32, tag=f"xta{t}", name=f"XTA{t}") for t in range(3)]
        YTA = [big.tile([96, RPC], f32, tag=f"yta{t}", name=f"YTA{t}") for t in range(3)]
        # stored savg (starts as prod of w, then Ln'd in place)
        Qst = [big.tile([128, N], f32, tag=f"qst{t}", name=f"Qst{t}") for t in range(NIB)]

        rsJ = [small.tile([128, H * NJB], f32, tag=f"rsj{t}", name=f"rsJ{t}") for t in range(NIB)]
        vdall = small.tile([128, NIB * H], f32)
        wdall = small.tile([128, NIB * H], f32)
        pd1 = small.tile([128, 16], f32)
        pd2 = small.tile([128, 8], f32)
        pdw = small.tile([128, 4], f32)
        Ldw = small.tile([128, 4], f32)
        rsc = small.tile([128, NIB * H], f32)
        Spart = small.tile([128, H], f32)
        stats = small.tile([128, NSTAT], f32)
        slq = small.tile([128, NIB], f32)
        ssig = small.tile([128, NIB], f32)
        scnt = small.tile([128, NIB], f32)
        ones128 = small.tile([128, 1], f32)
        ones1 = small.tile([1, 128], f32)
        half8 = small.tile([8, 1], f32)
        Sg = small.tile([1, H], f32)
        Ssum = small.tile([1, 1], f32)
        blavg_t = small.tile([1, 1], f32)
        nbl = small.tile([128, 1], f32)
        t8b = small.tile([128, 1], f32)
        outrow = small.tile([1, NSTAT], f32)

        nc.vector.memset(ones128[:], 1.0)
        nc.vector.memset(ones1[:], 1.0)
        nc.vector.memset(half8[:], 0.5)
        nc.vector.memset(stats[:], 0.0)

        # ---------- preprocessing: transposes + packed operand assembly ----------
        with (
            tc.tile_pool(name="pp_sbuf", bufs=4) as pp,
            tc.tile_pool(name="pp_keep", bufs=1) as ppk,
            tc.tile_pool(name="pp_psum", bufs=4, space="PSUM") as ppp,
        ):
            ident = ppk.tile([128, 128], f32)
            from concourse import masks

            masks.make_identity(nc, ident[:])

            X2T = ppk.tile([128, N], f32)  # -2 * zx^T
            YTfull = ppk.tile([128, RPC], f32)  # zy^T
            xna = ppk.tile([8, N], f32)  # xn_h[j] + 0.5
            yna = ppk.tile([8, RPC], f32)  # yn_h[i] + 0.5
            Hmask = ppk.tile([128, 8], f32)

            hm = np.zeros((128, 8), np.float32)
            for h in range(H):
                hm[h * DH : (h + 1) * DH, h] = 1.0
            hmd = nc.inline_tensor(hm, name="hmask_const")
            onesd = nc.inline_tensor(np.ones((1, N), np.float32), name="ones_const")

            # stage full inputs with ONE DMA each so PE transposes carry at
            # most one wait (PE transpose instructions only support a single
            # sync-wait command)
            SX = ppk.tile([128, N], f32)
            SYD = ppk.tile([128, NIB * 2 * D], f32)
            nc.gpsimd.dma_start(
                out=SX.rearrange("p (t d) -> p t d", d=D),
                in_=zx.rearrange("(t p) d -> p t d", p=128),
            )
            nc.gpsimd.dma_start(
                out=SYD.rearrange("p (t c) -> p t c", c=2 * D),
                in_=zyd.rearrange("(t p) c -> p t c", p=128),
            )
            nc.gpsimd.dma_start(out=Hmask[:], in_=hmd[:])

            def SY(t):
                return SYD[:, t * 2 * D : t * 2 * D + D]

            def SXD(t):
                return SYD[:, t * 2 * D + D : (t + 1) * 2 * D]

            # dummy transpose absorbs the identity-ready wait on PE; dummy
            # matmul absorbs the staging-DMA wait (all preproc DMAs share the
            # single SWDGE semaphore)
            pdum = ppp.tile([128, 128], f32, tag="tp")
            nc.tensor.transpose(pdum[:], ident[:], ident[:])
            pdm2 = ppp.tile([8, 8], f32, tag="xn")
            nc.tensor.matmul(out=pdm2[:], lhsT=Hmask[:, 0:8], rhs=Hmask[:, 0:8])
            for t in range(N // 128):
                pt = ppp.tile([128, 128], f32, tag="tp")
                nc.tensor.transpose(pt[:], SX[:, t * 128 : (t + 1) * 128], ident[:])
                nc.scalar.activation(
                    out=X2T[:, t * 128 : (t + 1) * 128], in_=pt[:], func=AF.Copy,
                    scale=-2.0,
                )
                sq = pp.tile([128, 128], f32, tag="sq")
                nc.scalar.activation(out=sq[:], in_=pt[:], func=AF.Square)
                xnp = ppp.tile([8, 128], f32, tag="xn")
                nc.tensor.matmul(out=xnp[:], lhsT=Hmask[:, 0:8], rhs=sq[:])
                nc.scalar.activation(
                    out=xna[:, t * 128 : (t + 1) * 128], in_=xnp[:],
                    func=AF.Identity, bias=half8[:], scale=1.0,
                )
            for t in range(RPC // 128):
                pt = ppp.tile([128, 128], f32, tag="tp")
                nc.tensor.transpose(pt[:], SY(t), ident[:])
                nc.scalar.activation(
                    out=YTfull[:, t * 128 : (t + 1) * 128], in_=pt[:], func=AF.Copy,
                )
                sq = pp.tile([128, 128], f32, tag="sq")
                nc.scalar.activation(out=sq[:], in_=pt[:], func=AF.Square)
                ynp = ppp.tile([8, 128], f32, tag="xn")
                nc.tensor.matmul(out=ynp[:], lhsT=Hmask[:, 0:8], rhs=sq[:])
                nc.scalar.activation(
                    out=yna[:, t * 128 : (t + 1) * 128], in_=ynp[:],
                    func=AF.Identity, bias=half8[:], scale=1.0,
                )

            # assemble packed operands
            # XTA rows: [32s,32s+16) = -2*zx_h^T ; 32s+16 = 1 ; 32s+17 = xn_h+0.5
            # YTA rows: [32s,32s+16) = zy_h^T    ; 32s+16 = yn_h+0.5 ; 32s+17 = 1
            for h in range(H):
                t, s = HT[h], HS[h]
                nc.gpsimd.dma_start(
                    out=XTA[t][32 * s : 32 * s + 16, :],
                    in_=X2T[DH * h : DH * (h + 1), :],
                )
                nc.gpsimd.dma_start(
                    out=XTA[t][32 * s + 16 : 32 * s + 17, :], in_=onesd[:]
                )
                nc.gpsimd.dma_start(
                    out=XTA[t][32 * s + 17 : 32 * s + 18, :], in_=xna[h : h + 1, :]
                )
                nc.gpsimd.dma_start(
                    out=YTA[t][32 * s : 32 * s + 16, :],
                    in_=YTfull[DH * h : DH * (h + 1), :],
                )
                nc.gpsimd.dma_start(
                    out=YTA[t][32 * s + 16 : 32 * s + 17, :], in_=yna[h : h + 1, :]
                )
                nc.gpsimd.dma_start(
                    out=YTA[t][32 * s + 17 : 32 * s + 18, :], in_=onesd[:, 0:RPC]
                )

            # diagonal path: vd_h(i) = 1 + ||zy_i - zx_i||^2 per head
            for t in range(NIB):
                dd = pp.tile([128, 128], f32, tag="dd")
                nc.vector.tensor_sub(dd[:], SY(t), SXD(t))
                nc.vector.tensor_mul(dd[:], dd[:], dd[:])
                nc.vector.tensor_reduce(
                    out=vdall[:, t * H : (t + 1) * H],
                    in_=dd.rearrange("p (h k) -> p h k", k=DH),
                    axis=AX.X, op=ALU.add,
                )
            nc.vector.tensor_scalar(
                out=vdall[:], in0=vdall[:], scalar1=1.0, scalar2=None, op0=ALU.add
            )
            nc.vector.reciprocal_approx_fast(out=wdall[:], in_=vdall[:])
            wv = wdall.rearrange("p (t c) -> p t c", c=8)
            nc.vector.tensor_mul(
                pd1.rearrange("p (t c) -> p t c", c=4), wv[:, :, 0:4], wv[:, :, 4:8]
            )
            p1v = pd1.rearrange("p (t c) -> p t c", c=4)
            nc.vector.tensor_mul(
                pd2.rearrange("p (t c) -> p t c", c=2), p1v[:, :, 0:2], p1v[:, :, 2:4]
            )
            p2v = pd2.rearrange("p (t c) -> p t c", c=2)
            nc.vector.tensor_mul(
                pdw.rearrange("p (t c) -> p t c", c=1), p2v[:, :, 0:1], p2v[:, :, 1:2]
            )

        # ---------- main loop ----------
        with (
            tc.tile_pool(name="mm_psum", bufs=2, space="PSUM") as mp,
            tc.tile_pool(name="wpool", bufs=3) as wp,
            tc.tile_pool(name="jpool", bufs=4) as jp,
            tc.tile_pool(name="upool", bufs=8) as up,
            tc.tile_pool(name="qpool", bufs=3) as qp,
        ):
            for ib in range(NIB):
                for jb in range(NJB):
                    wt = []
                    for r, heads in enumerate(RHEADS):
                        L = len(heads) * 512
                        ps = mp.tile([128, 1536], f32, tag="ps")
                        w = wp.tile([128, 1536], f32, tag="w")
                        # tiny write absorbs the WAR wait on this w slot so the
                        # reciprocal below only needs the PE wait (1-wait limit)
                        nc.vector.memset(w[0:1, 0:1], 0.0)
                        for si, h in enumerate(heads):
                            nc.tensor.matmul(
                                out=ps[:, si * 512 : (si + 1) * 512],
                                lhsT=YTA[r][32 * si : 32 * si + 18,
                                            ib * 128 : (ib + 1) * 128],
                                rhs=XTA[r][32 * si : 32 * si + 18,
                                           jb * 512 : (jb + 1) * 512],
                            )
                        nc.vector.reciprocal_approx_fast(
                            out=w[:, 0:L], in_=ps[:, 0:L]
                        )
                        for si, h in enumerate(heads):
                            jnk = jp.tile([128, 512], f32, tag="jnk")
                            col = h * NJB + jb
                            nc.scalar.activation(
                                out=jnk[:], in_=w[:, si * 512 : (si + 1) * 512],
                                func=AF.Copy,
                                accum_out=rsJ[ib][:, col : col + 1],
                            )
                        wt.append(w)
                    # product tree over the 8 heads: L1 on DVE, L2/L3 on GPSIMD
                    pairs = [
                        (wt[0][:, 0:512], wt[0][:, 512:1024]),      # h0*h1
                        (wt[0][:, 1024:1536], wt[1][:, 0:512]),     # h2*h3
                        (wt[1][:, 512:1024], wt[1][:, 1024:1536]),  # h4*h5
                        (wt[2][:, 0:512], wt[2][:, 512:1024]),      # h6*h7
                    ]
                    us = []
                    for pi, (a, b) in enumerate(pairs):
                        u = up.tile([128, 512], f32, tag="u", name=f"u{pi}")
                        nc.vector.tensor_mul(u[:], a, b)
                        us.append(u)
                    qa = qp.tile([128, 512], f32, tag="q")
                    qb = qp.tile([128, 512], f32, tag="q")
                    nc.gpsimd.tensor_mul(qa[:], us[0][:], us[1][:])
                    nc.gpsimd.tensor_mul(qb[:], us[2][:], us[3][:])
                    nc.gpsimd.tensor_mul(
                        Qst[ib][:, jb * 512 : (jb + 1) * 512], qa[:], qb[:]
                    )

        # ---------- finish: rowsums, collective, sweeps, pack ----------
        with (
            tc.tile_pool(name="fin_psum", bufs=1, space="PSUM") as fp,
            tc.tile_pool(name="fin_sbuf", bufs=2) as fs,
            tc.tile_pool(name="dram", bufs=1, space="DRAM") as dp,
        ):
            # per-row, per-head sums of w minus diagonal element
            for ib in range(NIB):
                rs8 = fs.tile([128, 8], f32, tag="rs8")
                nc.vector.tensor_reduce(
                    out=rs8[:], in_=rsJ[ib].rearrange("p (h j) -> p h j", j=NJB),
                    axis=AX.X, op=ALU.add,
                )
                nc.vector.tensor_sub(
                    rsc[:, ib * H : (ib + 1) * H], rs8[:],
                    wdall[:, ib * H : (ib + 1) * H],
                )
            # global per-head sums -> AllReduce
            nc.vector.tensor_reduce(
                out=Spart[:], in_=rsc.rearrange("p (t h) -> p h t", h=H),
                axis=AX.X, op=ALU.add,
            )
            psS = fp.tile([1, H], f32, tag="psS")
            nc.tensor.matmul(out=psS[:], lhsT=ones128[:, 0:1], rhs=Spart[:])
            ccin_sb = fs.tile([1, H], f32, tag="cc")
            nc.scalar.activation(out=ccin_sb[:], in_=psS[:], func=AF.Copy)
            cc_in = dp.tile([1, H], f32, tag="ccin")
            cc_out = dp.tile([1, H], f32, tag="ccout")
            nc.sync.dma_start(out=cc_in[:], in_=ccin_sb[:])
            nc.gpsimd.collective_compute(
                "AllReduce",
                mybir.AluOpType.add,
                replica_groups=[list(range(NCORES))],
                ins=[cc_in.opt()],
                outs=[cc_out.opt()],
            )
            nc.sync.dma_start(out=Sg[:], in_=cc_out[:])

            # blavg = mean_h ln(S_h) - ln(n(n-1)) ; broadcast to all partitions
            nc.scalar.activation(out=Sg[:], in_=Sg[:], func=AF.Ln)
            nc.vector.tensor_reduce(out=Ssum[:], in_=Sg[:], axis=AX.X, op=ALU.add)
            nc.scalar.activation(
                out=blavg_t[:], in_=Ssum[:], func=AF.Copy, scale=1.0 / H,
                bias=-LOG_NN1,
            )
            psB = fp.tile([128, 1], f32, tag="psB")
            nc.tensor.matmul(out=psB[:], lhsT=ones1[0:1, :], rhs=blavg_t[0:1, :])
            nc.scalar.activation(out=nbl[:], in_=psB[:], func=AF.Copy, scale=-1.0)
            nc.scalar.activation(out=t8b[:], in_=psB[:], func=AF.Copy, scale=float(H))

            # log sweep (savg = ln(prod w)) + sums
            for ib in range(NIB):
                nc.scalar.activation(out=Qst[ib][:], in_=Qst[ib][:], func=AF.Ln)
                nc.vector.tensor_reduce(
                    out=slq[:, ib : ib + 1], in_=Qst[ib][:], axis=AX.X, op=ALU.add
                )
            nc.scalar.activation(out=Ldw[:], in_=pdw[:], func=AF.Ln)
            nc.scalar.activation(out=rsc[:], in_=rsc[:], func=AF.Ln)

            # sigmoid + count sweeps (need blavg)
            for ib in range(NIB):
                sj = fs.tile([128, N], f32, tag="sj")
                nc.scalar.activation(
                    out=sj[:], in_=Qst[ib][:], func=AF.Sigmoid, scale=1.0 / H,
                    bias=nbl[:], accum_out=ssig[:, ib : ib + 1],
                )
                cj = fs.tile([128, N], f32, tag="cj")
                nc.vector.tensor_scalar(
                    out=cj[:], in0=Qst[ib][:], scalar1=t8b[:, 0:1], scalar2=None,
                    op0=ALU.is_gt, op1=ALU.add, accum_out=scnt[:, ib : ib + 1],
                )
            sigd = fs.tile([128, 4], f32, tag="sigd")
            sdtmp = fs.tile([128, 1], f32, tag="sdtmp")
            nc.scalar.activation(
                out=sigd[:], in_=Ldw[:], func=AF.Sigmoid, scale=1.0 / H,
                bias=nbl[:], accum_out=sdtmp[:],
            )
            nc.vector.tensor_copy(stats[:, 4:5], sdtmp[:])
            cd4 = fs.tile([128, 4], f32, tag="cd4")
            nc.vector.tensor_scalar(
                out=cd4[:], in0=Ldw[:], scalar1=t8b[:, 0:1], scalar2=None,
                op0=ALU.is_gt, op1=ALU.add, accum_out=stats[:, 5:6],
            )

            nc.vector.tensor_reduce(out=stats[:, 0:1], in_=Ldw[:], axis=AX.X, op=ALU.add)
            nc.vector.tensor_reduce(out=stats[:, 1:2], in_=slq[:], axis=AX.X, op=ALU.add)
            nc.vector.tensor_reduce(out=stats[:, 2:3], in_=ssig[:], axis=AX.X, op=ALU.add)
            nc.vector.tensor_reduce(out=stats[:, 3:4], in_=scnt[:], axis=AX.X, op=ALU.add)
            nc.vector.tensor_reduce(out=stats[:, 6:7], in_=rsc[:], axis=AX.X, op=ALU.add)

            psO = fp.tile([1, NSTAT], f32, tag="psO")
            nc.tensor.matmul(out=psO[:], lhsT=ones128[:, 0:1], rhs=stats[:])
            nc.scalar.activation(out=outrow[:], in_=psO[:], func=AF.Copy)
            nc.scalar.activation(
                out=outrow[:, 7:8], in_=blavg_t[:, 0:1], func=AF.Copy
            )
            nc.sync.dma_start(out=out[:], in_=outrow[:])

    nc.compile()
    return nc


_CACHED_NC = None


def _get_nc():
    global _CACHED_NC
    if _CACHED_NC is None:
        _CACHED_NC = build_bass()
    return _CACHED_NC


def make_in_maps(z_x, z_y):
    z_x = np.ascontiguousarray(z_x, dtype=np.float32)
    z_y = np.ascontiguousarray(z_y, dtype=np.float32)
    return [
        {
            "z_x": z_x,
            "z_yd": np.ascontiguousarray(
                np.concatenate(
                    [
                        z_y[c * RPC : (c + 1) * RPC],
                        z_x[c * RPC : (c + 1) * RPC],
                    ],
                    axis=1,
                )
            ),
        }
        for c in range(NCORES)
    ]


def combine(stats, z_x, z_y):
    """stats: [NCORES, NSTAT] float; returns the 9 reference outputs."""
    st = stats.astype(np.float64)
    blavg = float(st[0, 7])
    sum_Ld = st[:, 0].sum()  # sum_i sum_h l_h(i,i)
    sum_savg_full = st[:, 1].sum()
    sig_full = st[:, 2].sum()
    cnt_full = st[:, 3].sum()
    sig_diag = st[:, 4].sum()
    cp = st[:, 5].sum()
    rep_sum = st[:, 6].sum()

    mean_pos = sum_Ld / (H * N) - blavg
    mean_neg = (sum_savg_full - sum_Ld) / (H * N * (N - 1)) - blavg
    mean_sig_pos = sig_diag / N
    mean_sig_neg = (sig_full - sig_diag) / (N * (N - 1))
    cn = cnt_full - cp
    acc = (cp + (N * (N - 1) - cn)) / (N * N)
    recall = cp / N
    tpfp = cp + cn
    precision = (cp / max(tpfp, 1.0)) if tpfp > 0 else 0.0
    rep_mean = rep_sum / (H * N) - math.log(N - 1) - blavg
    zx64 = z_x.astype(np.float64)
    zy64 = z_y.astype(np.float64)
    decay = 0.01 * (np.mean(zx64 * zx64) + np.mean(zy64 * zy64))
    loss = -mean_pos + rep_mean + decay
    return np.array(
        [
            mean_pos, mean_neg, mean_sig_pos, mean_sig_neg, acc, recall,
            precision, blavg, loss,
        ],
        dtype=np.float32,
    )


def run_on_hw(z_x, z_y, trace=False):
    from concourse.bass_utils import run_bass_kernel_spmd

    nc = _get_nc()
    res = run_bass_kernel_spmd(
        nc, make_in_maps(z_x, z_y), core_ids=list(range(NCORES)), trace=trace
    )
    stats = np.stack([r["out"][0] for r in res.results])
    return combine(stats, z_x, z_y), res


def kernel(z_x, z_y):
    out, _ = run_on_hw(z_x, z_y, trace=False)
    return out


# revision 16
# speedup vs baseline: 1.3025x; 1.3025x over previous
"""Trainium2 Bass kernel for nn_MultiHeadDensityRatioEstimator.

Math restructure vs the jax reference:
  logits l_h(i,j) = -log1p(sq_h(i,j))  with sq = ||zy_i||^2+||zx_j||^2-2<zy_i,zx_j>
  exp(l_h) = 1/(1+sq_h) =: w_h   -> every logsumexp becomes a plain sum of w
  sum_h l_h = ln(prod_h w_h)     -> one log per pair instead of 8

Per core (8 cores, zy rows sharded 512/core):
  v_h = 1+sq_h from one K=18 augmented matmul per (head, tile)  [PSUM]
  w_h = reciprocal_approx_fast(v_h)                             [DVE]
  rowsums of w_h via ACT Copy+accum_out                         [ACT]
  savg = ln(prod_h w_h) stored [512,4096]                       [DVE/GPSIMD + ACT Ln]
  tiny AllReduce of the 8 per-head global sums -> baseline blavg
  sigmoid / count / sum sweeps over stored savg
  8 partial stats out per core; host combines to the 9 scalars.
"""

import math
import sys

import numpy as np

for _p in ("/opt/trn_rl_repo",):
    if _p not in sys.path:
        sys.path.insert(0, _p)

N = 4096
D = 128
H = 8
DH = 16
NCORES = 8
RPC = N // NCORES  # rows per core = 512
NIB = RPC // 128  # 4 i-blocks of 128 rows
NJB = N // 512  # 8 j-blocks of 512 cols
LOG_NN1 = float(np.log(float(N) * (N - 1)))
NSTAT = 8


def build_bass():
    import concourse.bacc as bacc
    import concourse.bass as bass
    import concourse.tile as tile
    from concourse import mybir

    f32 = mybir.dt.float32
    AF = mybir.ActivationFunctionType
    ALU = mybir.AluOpType
    AX = mybir.AxisListType

    nc = bacc.Bacc("TRN2", num_devices=NCORES, debug=False)

    zx = nc.dram_tensor("z_x", [N, D], f32, kind="ExternalInput")
    # z_yd[:, 0:128] = this core's zy rows; [:, 128:256] = matching zx rows
    zyd = nc.dram_tensor("z_yd", [RPC, 2 * D], f32, kind="ExternalInput")
    out = nc.dram_tensor("out", [1, NSTAT], f32, kind="ExternalOutput")

    from contextlib import ExitStack

    with tile.TileContext(nc) as tc, ExitStack() as stk:
        # ---------- persistent pools ----------
        big = stk.enter_context(tc.tile_pool(name="big", bufs=1))
        small = stk.enter_context(tc.tile_pool(name="small", bufs=1))

        # packed matmul operands: head h -> tensor HT[h], slot HS[h] (32-part
        # stride; matmul operand base partition must be 0/32/64)
        HT = [0, 0, 0, 1, 1, 1, 2, 2]
        HS = [0, 1, 2, 0, 1, 2, 0, 1]
        RHEADS = [[0, 1, 2], [3, 4, 5], [6, 7]]
        XTA = [big.tile([96, N], f32, tag=f"xta{t}", name=f"XTA{t}") for t in range(3)]
        YTA = [big.tile([96, RPC], f32, tag=f"yta{t}", name=f"YTA{t}") for t in range(3)]
        # stored savg (starts as prod of w, then Ln'd in place)
        Qst = [big.tile([128, N], f32, tag=f"qst{t}", name=f"Qst{t}") for t in range(NIB)]

        rsJ = [small.tile([128, H * NJB], f32, tag=f"rsj{t}", name=f"rsJ{t}") for t in range(NIB)]
        vdall = small.tile([128, NIB * H], f32)
        wdall = small.tile([128, NIB * H], f32)
        pd1 = small.tile([128, 16], f32)
        pd2 = small.tile([128, 8], f32)
        pdw = small.tile([128, 4], f32)
        Ldw = small.tile([128, 4], f32)
        rsc = small.tile([128, NIB * H], f32)
        Spart = small.tile([128, H], f32)
        stats = small.tile([128, NSTAT], f32)
        slq = small.tile([128, NIB], f32)
        ssig = small.tile([128, NIB], f32)
        scnt = small.tile([128, NIB], f32)
        ones128 = small.tile([128, 1], f32)
        ones1 = small.tile([1, 128], f32)
        half8 = small.tile([8, 1], f32)
        Sg = small.tile([1, H], f32)
        Ssum = small.tile([1, 1], f32)
        blavg_t = small.tile([1, 1], f32)
        nbl = small.tile([128, 1], f32)
        t8b = small.tile([128, 1], f32)
        outrow = small.tile([1, NSTAT], f32)

        nc.vector.memset(ones128[:], 1.0)
        nc.vector.memset(ones1[:], 1.0)
        nc.vector.memset(half8[:], 0.5)
        nc.vector.memset(stats[:], 0.0)

        # ---------- preprocessing: transposes + packed operand assembly ----------
        with (
            tc.tile_pool(name="pp_sbuf", bufs=4) as pp,
            tc.tile_pool(name="pp_keep", bufs=1) as ppk,
            tc.tile_pool(name="pp_psum", bufs=4, space="PSUM") as ppp,
        ):
            ident = ppk.tile([128, 128], f32)
            from concourse import masks

            masks.make_identity(nc, ident[:])

            X2T = ppk.tile([128, N], f32)  # -2 * zx^T
            YTfull = ppk.tile([128, RPC], f32)  # zy^T
            xna = ppk.tile([8, N], f32)  # xn_h[j] + 0.5
            yna = ppk.tile([8, RPC], f32)  # yn_h[i] + 0.5
            Hmask = ppk.tile([128, 8], f32)

            hm = np.zeros((128, 8), np.float32)
            for h in range(H):
                hm[h * DH : (h + 1) * DH, h] = 1.0
            hmd = nc.inline_tensor(hm, name="hmask_const")
            onesd = nc.inline_tensor(np.ones((1, N), np.float32), name="ones_const")

            # stage full inputs with ONE DMA each so PE transposes carry at
            # most one wait (PE transpose instructions only support a single
            # sync-wait command)
            SX = ppk.tile([128, N], f32)
            SYD = ppk.tile([128, NIB * 2 * D], f32)
            nc.gpsimd.dma_start(
                out=SX.rearrange("p (t d) -> p t d", d=D),
                in_=zx.rearrange("(t p) d -> p t d", p=128),
            )
            nc.gpsimd.dma_start(
                out=SYD.rearrange("p (t c) -> p t c", c=2 * D),
                in_=zyd.rearrange("(t p) c -> p t c", p=128),
            )
            nc.gpsimd.dma_start(out=Hmask[:], in_=hmd[:])

            def SY(t):
                return SYD[:, t * 2 * D : t * 2 * D + D]

            def SXD(t):
                return SYD[:, t * 2 * D + D : (t + 1) * 2 * D]

            # dummy transpose absorbs the identity-ready wait on PE; dummy
            # matmul absorbs the staging-DMA wait (all preproc DMAs share the
            # single SWDGE semaphore)
            pdum = ppp.tile([128, 128], f32, tag="tp")
            nc.tensor.transpose(pdum[:], ident[:], ident[:])
            pdm2 = ppp.tile([8, 8], f32, tag="xn")
            nc.tensor.matmul(out=pdm2[:], lhsT=Hmask[:, 0:8], rhs=Hmask[:, 0:8])
            for t in range(N // 128):
                pt = ppp.tile([128, 128], f32, tag="tp")
                nc.tensor.transpose(pt[:], SX[:, t * 128 : (t + 1) * 128], ident[:])
                nc.scalar.activation(
                    out=X2T[:, t * 128 : (t + 1) * 128], in_=pt[:], func=AF.Copy,
                    scale=-2.0,
                )
                sq = pp.tile([128, 128], f32, tag="sq")
                nc.scalar.activation(out=sq[:], in_=pt[:], func=AF.Square)
                xnp = ppp.tile([8, 128], f32, tag="xn")
                nc.tensor.matmul(out=xnp[:], lhsT=Hmask[:, 0:8], rhs=sq[:])
                nc.scalar.activation(
                    out=xna[:, t * 128 : (t + 1) * 128], in_=xnp[:],
                    func=AF.Identity, bias=half8[:], scale=1.0,
                )
            for t in range(RPC // 128):
                pt = ppp.tile([128, 128], f32, tag="tp")
                nc.tensor.transpose(pt[:], SY(t), ident[:])
                nc.scalar.activation(
                    out=YTfull[:, t * 128 : (t + 1) * 128], in_=pt[:], func=AF.Copy,
                )
                sq = pp.tile([128, 128], f32, tag="sq")
                nc.scalar.activation(out=sq[:], in_=pt[:], func=AF.Square)
                ynp = ppp.tile([8, 128], f32, tag="xn")
                nc.tensor.matmul(out=ynp[:], lhsT=Hmask[:, 0:8], rhs=sq[:])
                nc.scalar.activation(
                    out=yna[:, t * 128 : (t + 1) * 128], in_=ynp[:],
                    func=AF.Identity, bias=half8[:], scale=1.0,
                )

            # assemble packed operands
            # XTA rows: [32s,32s+16) = -2*zx_h^T ; 32s+16 = 1 ; 32s+17 = xn_h+0.5
            # YTA rows: [32s,32s+16) = zy_h^T    ; 32s+16 = yn_h+0.5 ; 32s+17 = 1
            for h in range(H):
                t, s = HT[h], HS[h]
                nc.gpsimd.dma_start(
                    out=XTA[t][32 * s : 32 * s + 16, :],
                    in_=X2T[DH * h : DH * (h + 1), :],
                )
                nc.gpsimd.dma_start(
                    out=XTA[t][32 * s + 16 : 32 * s + 17, :], in_=onesd[:]
                )
                nc.gpsimd.dma_start(
                    out=XTA[t][32 * s + 17 : 32 * s + 18, :], in_=xna[h : h + 1, :]
                )
                nc.gpsimd.dma_start(
                    out=YTA[t][32 * s : 32 * s + 16, :],
                    in_=YTfull[DH * h : DH * (h + 1), :],
                )
                nc.gpsimd.dma_start(
                    out=YTA[t][32 * s + 16 : 32 * s + 17, :], in_=yna[h : h + 1, :]
                )
                nc.gpsimd.dma_start(
                    out=YTA[t][32 * s + 17 : 32 * s + 18, :], in_=onesd[:, 0:RPC]
                )

            # diagonal path: vd_h(i) = 1 + ||zy_i - zx_i||^2 per head
            for t in range(NIB):
                dd = pp.tile([128, 128], f32, tag="dd")
                nc.vector.tensor_sub(dd[:], SY(t), SXD(t))
                nc.vector.tensor_mul(dd[:], dd[:], dd[:])
                nc.vector.tensor_reduce(
                    out=vdall[:, t * H : (t + 1) * H],
                    in_=dd.rearrange("p (h k) -> p h k", k=DH),
                    axis=AX.X, op=ALU.add,
                )
            nc.vector.tensor_scalar(
                out=vdall[:], in0=vdall[:], scalar1=1.0, scalar2=None, op0=ALU.add
            )
            nc.vector.reciprocal_approx_fast(out=wdall[:], in_=vdall[:])
            wv = wdall.rearrange("p (t c) -> p t c", c=8)
            nc.vector.tensor_mul(
                pd1.rearrange("p (t c) -> p t c", c=4), wv[:, :, 0:4], wv[:, :, 4:8]
            )
            p1v = pd1.rearrange("p (t c) -> p t c", c=4)
            nc.vector.tensor_mul(
                pd2.rearrange("p (t c) -> p t c", c=2), p1v[:, :, 0:2], p1v[:, :, 2:4]
            )
            p2v = pd2.rearrange("p (t c) -> p t c", c=2)
            nc.vector.tensor_mul(
                pdw.rearrange("p (t c) -> p t c", c=1), p2v[:, :, 0:1], p2v[:, :, 1:2]
            )

        # ---------- main loop ----------
        with (
            tc.tile_pool(name="mm_psum", bufs=2, space="PSUM") as mp,
            tc.tile_pool(name="wpool", bufs=3) as wp,
            tc.tile_pool(name="wpool2", bufs=3) as wp2,
            tc.tile_pool(name="jpool", bufs=4) as jp,
            tc.tile_pool(name="upool", bufs=8) as up,
            tc.tile_pool(name="qpool", bufs=3) as qp,
        ):
            bf16 = mybir.dt.bfloat16
            for ib in range(NIB):
                for jb in range(NJB):
                    w2t = []
                    for r, heads in enumerate(RHEADS):
                        L = len(heads) * 512
                        ps = mp.tile([128, 1536], f32, tag="ps")
                        w = wp.tile([128, 1536], f32, tag="w")
                        w2 = wp2.tile([128, 1536], bf16, tag="w2")
                        # tiny write absorbs the WAR wait on this w slot so the
                        # reciprocal below only needs the PE wait (1-wait limit)
                        nc.vector.memset(w[0:1, 0:1], 0.0)
                        for si, h in enumerate(heads):
                            nc.tensor.matmul(
                                out=ps[:, si * 512 : (si + 1) * 512],
                                lhsT=YTA[r][32 * si : 32 * si + 18,
                                            ib * 128 : (ib + 1) * 128],
                                rhs=XTA[r][32 * si : 32 * si + 18,
                                           jb * 512 : (jb + 1) * 512],
                            )
                        nc.vector.reciprocal_approx_fast(
                            out=w[:, 0:L], in_=ps[:, 0:L]
                        )
                        # rowsum + bf16 copy in one op: rounds 0/1 on ACT,
                        # round 2 on GpSimd (1-input ops run near line rate)
                        for si, h in enumerate(heads):
                            col = h * NJB + jb
                            nc.scalar.activation(
                                out=w2[:, si * 512 : (si + 1) * 512],
                                in_=w[:, si * 512 : (si + 1) * 512],
                                func=AF.Copy,
                                accum_out=rsJ[ib][:, col : col + 1],
                            )
                        w2t.append(w2)
                    # product tree over the 8 heads in bf16 (2x DVE rate):
                    # L1 on DVE, L2/L3 on GPSIMD, final write fp32
                    pairs = [
                        (w2t[0][:, 0:512], w2t[0][:, 512:1024]),      # h0*h1
                        (w2t[0][:, 1024:1536], w2t[1][:, 0:512]),     # h2*h3
                        (w2t[1][:, 512:1024], w2t[1][:, 1024:1536]),  # h4*h5
                        (w2t[2][:, 0:512], w2t[2][:, 512:1024]),      # h6*h7
                    ]
                    us = []
                    for pi, (a, b) in enumerate(pairs):
                        u = up.tile([128, 512], bf16, tag="u", name=f"u{pi}")
                        nc.vector.tensor_mul(u[:], a, b)
                        us.append(u)
                    qa = qp.tile([128, 512], bf16, tag="q")
                    qb = qp.tile([128, 512], bf16, tag="q")
                    nc.gpsimd.tensor_mul(qa[:], us[0][:], us[1][:])
                    nc.gpsimd.tensor_mul(qb[:], us[2][:], us[3][:])
                    nc.gpsimd.tensor_mul(
                        Qst[ib][:, jb * 512 : (jb + 1) * 512], qa[:], qb[:]
                    )

        # ---------- finish: rowsums, collective, sweeps, pack ----------
        with (
            tc.tile_pool(name="fin_psum", bufs=1, space="PSUM") as fp,
            tc.tile_pool(name="fin_sbuf", bufs=2) as fs,
            tc.tile_pool(name="dram", bufs=1, space="DRAM") as dp,
        ):
            # per-row, per-head sums of w minus diagonal element
            for ib in range(NIB):
                rs8 = fs.tile([128, 8], f32, tag="rs8")
                nc.vector.tensor_reduce(
                    out=rs8[:], in_=rsJ[ib].rearrange("p (h j) -> p h j", j=NJB),
                    axis=AX.X, op=ALU.add,
                )
                nc.vector.tensor_sub(
                    rsc[:, ib * H : (ib + 1) * H], rs8[:],
                    wdall[:, ib * H : (ib + 1) * H],
                )
            # global per-head sums -> AllReduce
            nc.vector.tensor_reduce(
                out=Spart[:], in_=rsc.rearrange("p (t h) -> p h t", h=H),
                axis=AX.X, op=ALU.add,
            )
            psS = fp.tile([1, H], f32, tag="psS")
            nc.tensor.matmul(out=psS[:], lhsT=ones128[:, 0:1], rhs=Spart[:])
            ccin_sb = fs.tile([1, H], f32, tag="cc")
            nc.scalar.activation(out=ccin_sb[:], in_=psS[:], func=AF.Copy)
            cc_in = dp.tile([1, H], f32, tag="ccin")
            cc_out = dp.tile([1, H], f32, tag="ccout")
            nc.sync.dma_start(out=cc_in[:], in_=ccin_sb[:])
            nc.gpsimd.collective_compute(
                "AllReduce",
                mybir.AluOpType.add,
                replica_groups=[list(range(NCORES))],
                ins=[cc_in.opt()],
                outs=[cc_out.opt()],
            )
            nc.sync.dma_start(out=Sg[:], in_=cc_out[:])

            # blavg = mean_h ln(S_h) - ln(n(n-1)) ; broadcast to all partitions
            nc.scalar.activation(out=Sg[:], in_=Sg[:], func=AF.Ln)
            nc.vector.tensor_reduce(out=Ssum[:], in_=Sg[:], axis=AX.X, op=ALU.add)
            nc.scalar.activation(
                out=blavg_t[:], in_=Ssum[:], func=AF.Copy, scale=1.0 / H,
                bias=-LOG_NN1,
            )
            psB = fp.tile([128, 1], f32, tag="psB")
            nc.tensor.matmul(out=psB[:], lhsT=ones1[0:1, :], rhs=blavg_t[0:1, :])
            nc.scalar.activation(out=nbl[:], in_=psB[:], func=AF.Copy, scale=-1.0)
            nc.scalar.activation(out=t8b[:], in_=psB[:], func=AF.Copy, scale=float(H))

            # log sweep (savg = ln(prod w)) + sums
            for ib in range(NIB):
                nc.scalar.activation(out=Qst[ib][:], in_=Qst[ib][:], func=AF.Ln)
                nc.vector.tensor_reduce(
                    out=slq[:, ib : ib + 1], in_=Qst[ib][:], axis=AX.X, op=ALU.add
                )
            nc.scalar.activation(out=Ldw[:], in_=pdw[:], func=AF.Ln)
            nc.scalar.activation(out=rsc[:], in_=rsc[:], func=AF.Ln)

            # sigmoid + count sweeps (need blavg)
            for ib in range(NIB):
                sj = fs.tile([128, N], f32, tag="sj")
                nc.scalar.activation(
                    out=sj[:], in_=Qst[ib][:], func=AF.Sigmoid, scale=1.0 / H,
                    bias=nbl[:], accum_out=ssig[:, ib : ib + 1],
                )
                cj = fs.tile([128, N], f32, tag="cj")
                nc.vector.tensor_scalar(
                    out=cj[:], in0=Qst[ib][:], scalar1=t8b[:, 0:1], scalar2=None,
                    op0=ALU.is_gt, op1=ALU.add, accum_out=scnt[:, ib : ib + 1],
                )
            sigd = fs.tile([128, 4], f32, tag="sigd")
            sdtmp = fs.tile([128, 1], f32, tag="sdtmp")
            nc.scalar.activation(
                out=sigd[:], in_=Ldw[:], func=AF.Sigmoid, scale=1.0 / H,
                bias=nbl[:], accum_out=sdtmp[:],
            )
            nc.vector.tensor_copy(stats[:, 4:5], sdtmp[:])
            cd4 = fs.tile([128, 4], f32, tag="cd4")
            nc.vector.tensor_scalar(
                out=cd4[:], in0=Ldw[:], scalar1=t8b[:, 0:1], scalar2=None,
                op0=ALU.is_gt, op1=ALU.add, accum_out=stats[:, 5:6],
            )

            nc.vector.tensor_reduce(out=stats[:, 0:1], in_=Ldw[:], axis=AX.X, op=ALU.add)
            nc.vector.tensor_reduce(out=stats[:, 1:2], in_=slq[:], axis=AX.X, op=ALU.add)
            nc.vector.tensor_reduce(out=stats[:, 2:3], in_=ssig[:], axis=AX.X, op=ALU.add)
            nc.vector.tensor_reduce(out=stats[:, 3:4], in_=scnt[:], axis=AX.X, op=ALU.add)
            nc.vector.tensor_reduce(out=stats[:, 6:7], in_=rsc[:], axis=AX.X, op=ALU.add)

            psO = fp.tile([1, NSTAT], f32, tag="psO")
            nc.tensor.matmul(out=psO[:], lhsT=ones128[:, 0:1], rhs=stats[:])
            nc.scalar.activation(out=outrow[:], in_=psO[:], func=AF.Copy)
            nc.scalar.activation(
                out=outrow[:, 7:8], in_=blavg_t[:, 0:1], func=AF.Copy
            )
            nc.sync.dma_start(out=out[:], in_=outrow[:])

    nc.compile()
    return nc


_CACHED_NC = None


def _get_nc():
    global _CACHED_NC
    if _CACHED_NC is None:
        _CACHED_NC = build_bass()
    return _CACHED_NC


def make_in_maps(z_x, z_y):
    z_x = np.ascontiguousarray(z_x, dtype=np.float32)
    z_y = np.ascontiguousarray(z_y, dtype=np.float32)
    return [
        {
            "z_x": z_x,
            "z_yd": np.ascontiguousarray(
                np.concatenate(
                    [
                        z_y[c * RPC : (c + 1) * RPC],
                        z_x[c * RPC : (c + 1) * RPC],
                    ],
                    axis=1,
                )
            ),
        }
        for c in range(NCORES)
    ]


def combine(stats, z_x, z_y):
    """stats: [NCORES, NSTAT] float; returns the 9 reference outputs."""
    st = stats.astype(np.float64)
    blavg = float(st[0, 7])
    sum_Ld = st[:, 0].sum()  # sum_i sum_h l_h(i,i)
    sum_savg_full = st[:, 1].sum()
    sig_full = st[:, 2].sum()
    cnt_full = st[:, 3].sum()
    sig_diag = st[:, 4].sum()
    cp = st[:, 5].sum()
    rep_sum = st[:, 6].sum()

    mean_pos = sum_Ld / (H * N) - blavg
    mean_neg = (sum_savg_full - sum_Ld) / (H * N * (N - 1)) - blavg
    mean_sig_pos = sig_diag / N
    mean_sig_neg = (sig_full - sig_diag) / (N * (N - 1))
    cn = cnt_full - cp
    acc = (cp + (N * (N - 1) - cn)) / (N * N)
    recall = cp / N
    tpfp = cp + cn
    precision = (cp / max(tpfp, 1.0)) if tpfp > 0 else 0.0
    rep_mean = rep_sum / (H * N) - math.log(N - 1) - blavg
    zx64 = z_x.astype(np.float64)
    zy64 = z_y.astype(np.float64)
    decay = 0.01 * (np.mean(zx64 * zx64) + np.mean(zy64 * zy64))
    loss = -mean_pos + rep_mean + decay
    return np.array(
        [
            mean_pos, mean_neg, mean_sig_pos, mean_sig_neg, acc, recall,
            precision, blavg, loss,
        ],
        dtype=np.float32,
    )


def run_on_hw(z_x, z_y, trace=False):
    from concourse.bass_utils import run_bass_kernel_spmd

    nc = _get_nc()
    res = run_bass_kernel_spmd(
        nc, make_in_maps(z_x, z_y), core_ids=list(range(NCORES)), trace=trace
    )
    stats = np.stack([r["out"][0] for r in res.results])
    return combine(stats, z_x, z_y), res


def kernel(z_x, z_y):
    out, _ = run_on_hw(z_x, z_y, trace=False)
    return out
